# revision 25
# baseline (speedup 1.0000x reference)
"""Trainium2 Bass kernel for nn_CondScoreModelGNN (8-core SPMD).

Graph structure exploited: dst = tile(arange(N), 10) -> every node receives
exactly 10 edges; segment_max becomes 10 regular max-accumulation rounds.

Sharding: nodes (padded 50000->50176) split into 8 contiguous shards of 6272.
Each core computes its shard's conditioning + init features, builds its shard
of the EdgeConv neighbor table hn = x @ Wn ROW-major in fp16, AllGathers the
table (strided out-AP lands shards 0-3 / 4-7 in two half-tables of 25088 rows
each so rows are int16-indexable), then processes its shard's edges with
dma_gather(transpose=True): one instruction gathers+transposes 6272 rows per
round per half-table; dead slots point at a guard zero row so the two half
gathers can simply be summed. fp16 operands give 1 cycle/row matmuls.
"""
import sys

sys.path.insert(0, "/opt/trn_rl_repo")

import numpy as np

N_CORES = 8
N = 50000
E = 500000
B = 1024
H = 128
EM = 64
CLS = 10
KDEG = 10          # edges per node
NPAD = 50176       # 8 * 6272
SH = 6272          # nodes per core
NBLK = 49          # 128-node blocks per core
HALF = 25088       # rows per half-table (= 4 shards)
GROW = HALF        # guard zero row index within a half-table
IDXC = SH // 16    # idx columns per round (392)
TWO_PI = 2.0 * np.pi
SIGMA = 25.0
LOG_SIGMA = float(np.log(SIGMA))

# node chunks of 512 (12 full + 1 tail of 128)
CHUNKS = [(i * 512, 512) for i in range(12)] + [(6144, 128)]
NCH = len(CHUNKS)

_CACHE = {}


def _split_multi_waits(nc, mybir):
    """This walrus build encodes at most one sync wait per TPB_CTRL
    instruction; hoist extra waits into single-wait EventSemaphore insts."""
    n_split = 0
    for fn in nc.m.functions:
        for bb in fn.blocks:
            insts = list(bb.instructions)
            out = []
            changed = False
            for ins in insts:
                si = ins.sync_info
                waits = list(si.on_wait) if (si is not None and si.on_wait) else []
                is_drain = type(ins).__name__ in ("InstDrain", "InstDMAGatherAnt")
                if (len(waits) > 1) or (is_drain and len(waits) > 0):
                    changed = True
                    n_split += 1
                    for w in waits:
                        ev = mybir.InstEventSemaphore(
                            name=nc.get_next_instruction_name(),
                            opcode="EventSemaphore",
                            engine=ins.engine,
                            ins=[],
                            outs=[],
                            sync_info=mybir.SyncInfo(on_wait=[w], on_update=[]),
                        )
                        nc.register_instruction(ev)
                        out.append(ev)
                    si.on_wait = []
                    ins.sync_info = si
                out.append(ins)
            if changed:
                bb.instructions = out
    return n_split


def _build(debug=False, iters=1):
    import concourse.bass as bass
    import concourse.tile as tile
    from concourse import mybir
    from concourse.masks import make_identity

    f32 = mybir.dt.float32
    fp16 = mybir.dt.float16
    i32 = mybir.dt.int32
    AF = mybir.ActivationFunctionType
    ALU = mybir.AluOpType

    nc = bass.Bass()

    # ---------------- I/O ----------------
    bandb_in = nc.dram_tensor("bandb", [97, SH], fp16, kind="ExternalInput")
    bandt_in = nc.dram_tensor("bandt", [1, SH], f32, kind="ExternalInput")
    gfpm_in = nc.dram_tensor("gfpm", [64, SH], fp16, kind="ExternalInput")
    sidx_d = nc.dram_tensor("sidx", [128, KDEG * NBLK], i32,
                            kind="ExternalInput")
    smb_d = nc.dram_tensor("smb", [97, 256], fp16, kind="ExternalInput")
    btile_d = nc.dram_tensor("btile", [128, 16], f32, kind="ExternalInput")
    wnames = [
        ("i2", [128, 128]), ("sW", [64, 64]), ("w2", [128, 64]),
        ("m1W2", [128, 128]), ("m2W2", [128, 32]),
        ("m1cA", [128, 128]), ("m1cB", [128, 128]), ("m1cC", [64, 128]),
        ("m1nA", [128, 128]), ("m1nB", [128, 128]), ("m1nC", [64, 128]),
        ("m2cA", [128, 128]), ("m2cB", [128, 128]), ("m2cC", [64, 128]),
        ("m2nA", [128, 128]), ("m2nB", [128, 128]), ("m2nC", [64, 128]),
    ]
    wdram = {n: nc.dram_tensor(n, s, fp16, kind="ExternalInput")
             for n, s in wnames}
    y_out = nc.dram_tensor("y", [128, 1664], f32, kind="ExternalOutput")
    dbg = {}
    if debug:
        for nm, sh, dt_ in [("dxT1", [128, SH], fp16),
                            ("dxT2", [64, SH], fp16),
                            ("dhcb", [128, SH], fp16),
                            ("dout1", [128, SH], fp16),
                            ("dtabL", [128, 128], fp16),
                            ("dtabH", [128, 128], fp16),
                            ("dacc2", [128, 1664], f32),
                            ("drcp", [1, SH], fp16)]:
            dbg[nm] = nc.dram_tensor(nm, sh, dt_, kind="ExternalOutput")

    # collective bounce buffers: in = local shard (row-major fp16),
    # out = the full gathered table.
    inb = [nc.dram_tensor(f"inb{v}", [SH, 128], fp16) for v in range(2)]
    tabs = [
        nc.dram_tensor(f"tab{v}", [NPAD, 128], fp16, addr_space="Shared")
        for v in range(2)
    ]

    RG = [list(range(N_CORES))]

    with tile.TileContext(nc) as tc:
        with (
            tc.tile_pool(name="wpool", bufs=1) as wpool,
            tc.tile_pool(name="npool", bufs=1) as npool,
            tc.tile_pool(name="psA", bufs=3, space="PSUM") as psA,
            tc.tile_pool(name="psT", bufs=2, space="PSUM") as psT,
            tc.tile_pool(name="psO", bufs=3, space="PSUM") as psO,
        ):
            # ---------- persistent SBUF ----------
            bandb = npool.tile([97, SH], fp16)     # xin 0:6, onehot 32:42, wall 64:66, rcp 96
            bandt = npool.tile([1, SH], f32)       # t
            gfpm = npool.tile([64, SH], fp16)      # centered frac of t*w (+1/4)
            xT0 = npool.tile([128, SH], fp16)      # init -> (conv2) out1
            xT1 = npool.tile([128, SH], fp16)      # class(0:64) + sigma(64:128)
            xT2 = npool.tile([64, SH], fp16)       # wall_feat
            hcb = npool.tile([128, SH], fp16)      # center term + b1 (per conv)
            acc = npool.tile([128, SH], f32)       # conv1 max accum
            acc2v = npool.tile([128, 1664], f32)   # conv2 packed accum -> y
            sidx = npool.tile([128, KDEG * NBLK], i32)
            ident = wpool.tile([128, 128], f32)
            identh = wpool.tile([128, 128], fp16)

            smb = wpool.tile([97, 256], fp16)
            bt = wpool.tile([128, 16], f32)
            wt = {n: wpool.tile(s, fp16, name=f"wt_{n}") for n, s in wnames}

            nc.sync.dma_start(out=bandb[:], in_=bandb_in[:, :])
            nc.sync.dma_start(out=bandt[:], in_=bandt_in[:, :])
            nc.sync.dma_start(out=gfpm[:], in_=gfpm_in[:, :])
            nc.sync.dma_start(out=sidx[:], in_=sidx_d[:, :])
            nc.sync.dma_start(out=smb[:], in_=smb_d[:, :])
            nc.sync.dma_start(out=bt[:], in_=btile_d[:, :])
            for n, s in wnames:
                nc.sync.dma_start(out=wt[n][:], in_=wdram[n][:, :])
            make_identity(nc, ident[:])
            nc.vector.tensor_copy(out=identh[:], in_=ident[:])

            def mm(out_ap, lhsT_ap, rhs_ap, start=True, stop=True,
                   tile_position=None):
                nc.tensor.matmul(
                    out_ap,
                    lhsT_ap,
                    rhs_ap,
                    start=start,
                    stop=stop,
                    tile_position=tile_position,
                )

            # bias column APs
            ib1 = bt[:, 0:1]
            ib2 = bt[:, 1:2]
            b_cs = bt[:, 2:3]        # [0:64]=0 (class), [64:128]=sb
            wb1c = bt[64:128, 3:4]
            wb2 = bt[0:64, 4:5]
            m1b1 = bt[:, 5:6]
            m1b2 = bt[:, 6:7]
            m2b1 = bt[:, 7:8]
            m2b2v = bt[:, 8:9]       # m2b2 replicated at 32q+i

            for it_ in range(iters):
                # ---------- phase A: node features + conv1 table ----------
                with (
                    tc.tile_pool(name=f"stg{it_}", bufs=2) as stg,
                ):
                    for cs, cw in CHUNKS:
                        csl = slice(cs, cs + cw)
                        nb = cw // 128
                        # init MLP
                        p1 = psA.tile([128, 512], f32, tag="pa")
                        mm(p1[:, :cw], smb[0:6, 0:128], bandb[0:6, csl])
                        i1s = stg.tile([128, 512], fp16, tag="i1s")
                        nc.scalar.activation(i1s[:, :cw], p1[:, :cw], AF.Relu,
                                             bias=ib1)
                        p2 = psA.tile([128, 512], f32, tag="pa")
                        mm(p2[:, :cw], wt["i2"][:], i1s[:, :cw])
                        nc.scalar.activation(xT0[:, csl], p2[:, :cw], AF.Relu,
                                             bias=ib2)

                        # wall layer-1 pre-act -> p3[64:128]
                        p3 = psA.tile([128, 512], f32, tag="pa")
                        mm(p3[64:128, :cw], smb[64:66, 64:128],
                           bandb[64:66, csl])
                        # gfp: sin of host-range-reduced centered phases
                        gfps = stg.tile([64, 512], fp16, tag="gfps")
                        nc.scalar.activation(gfps[0:64, :cw], gfpm[0:64, csl],
                                             AF.Sin, scale=TWO_PI)
                        whs = stg.tile([128, 512], fp16, tag="whs")
                        nc.scalar.activation(whs[64:128, :cw], p3[64:128, :cw],
                                             AF.Relu, bias=wb1c)
                        # class (psum [0:64]) + sigma (psum [64:128])
                        p4 = psA.tile([128, 512], f32, tag="pa")
                        mm(p4[0:64, :cw], smb[32:42, 0:64],
                           bandb[32:42, csl])
                        mm(p4[64:128, :cw], wt["sW"][:], gfps[:, :cw])
                        nc.scalar.activation(xT1[:, csl], p4[:, :cw], AF.Relu,
                                             bias=b_cs)
                        # wall MLP layer 2
                        p6 = psA.tile([128, 512], f32, tag="pa")
                        mm(p6[0:64, :cw], wt["w2"][64:128, :],
                           whs[64:128, :cw])
                        nc.scalar.activation(xT2[:, csl], p6[0:64, :cw],
                                             AF.Relu, bias=wb2)

                        # conv1 neighbor table rows (row-major [node, feat])
                        pt = psA.tile([128, 512], f32, tag="pa")
                        for b in range(nb):
                            bsl = slice(cs + 128 * b, cs + 128 * (b + 1))
                            osl = slice(128 * b, 128 * (b + 1))
                            mm(pt[:, osl], xT0[:, bsl], wt["m1nA"][:],
                               start=True, stop=False)
                            mm(pt[:, osl], xT1[:, bsl], wt["m1nB"][:],
                               start=False, stop=False)
                            mm(pt[:, osl], xT2[:, bsl], wt["m1nC"][:],
                               start=False, stop=True)
                        hnr = stg.tile([128, 512], fp16, tag="hnr")
                        nc.vector.tensor_copy(out=hnr[:, :cw], in_=pt[:, :cw])
                        dst_ap = inb[0][csl, :].rearrange(
                            "(b p) f -> p b f", p=128)
                        src_ap = hnr[:].rearrange(
                            "p (b f) -> p b f", f=128)[:, :nb, :]
                        nc.sync.dma_start(out=dst_ap, in_=src_ap)

                nc.gpsimd.collective_compute(
                    "AllGather", ALU.bypass, replica_groups=RG,
                    ins=[inb[0].ap().opt()],
                    outs=[tabs[0].ap().opt()],
                )

                nc.gpsimd.memset(acc[:], -3.0e38)
                nc.gpsimd.memset(acc2v[:], -3.0e38)

                # ---------- helpers ----------
                def build_hcb(conv):
                    wa, wb, wc = (f"m{conv}cA", f"m{conv}cB", f"m{conv}cC")
                    b1 = m1b1 if conv == 1 else m2b1
                    for cs, cw in CHUNKS:
                        csl = slice(cs, cs + cw)
                        ph = psA.tile([128, 512], f32, tag="pa")
                        mm(ph[:, :cw], wt[wa][:], xT0[:, csl],
                           start=True, stop=False)
                        mm(ph[:, :cw], wt[wb][:], xT1[:, csl],
                           start=False, stop=False)
                        mm(ph[:, :cw], wt[wc][:], xT2[:, csl],
                           start=False, stop=True)
                        nc.scalar.activation(hcb[:, csl], ph[:, :cw],
                                             AF.Identity, bias=b1)

                def build_table2():
                    for cs, cw in CHUNKS:
                        nb = cw // 128
                        pt = psA.tile([128, 512], f32, tag="pa")
                        for b in range(nb):
                            bsl = slice(cs + 128 * b, cs + 128 * (b + 1))
                            osl = slice(128 * b, 128 * (b + 1))
                            mm(pt[:, osl], xT0[:, bsl], wt["m2nA"][:],
                               start=True, stop=False)
                            mm(pt[:, osl], xT1[:, bsl], wt["m2nB"][:],
                               start=False, stop=False)
                            mm(pt[:, osl], xT2[:, bsl], wt["m2nC"][:],
                               start=False, stop=True)
                        hnr = t2p.tile([128, 512], fp16, tag="hnr2")
                        nc.vector.tensor_copy(out=hnr[:, :cw], in_=pt[:, :cw])
                        dst_ap = inb[1][cs:cs + cw, :].rearrange(
                            "(b p) f -> p b f", p=128)
                        src_ap = hnr[:].rearrange(
                            "p (b f) -> p b f", f=128)[:, :nb, :]
                        nc.sync.dma_start(out=dst_ap, in_=src_ap)

                def gath(gp, tab, r, ci, cw):
                    nb = cw // 128
                    gd = gp.tile([128, 4, 128], fp16, tag="gd")
                    for b in range(nb):
                        q = NBLK * r + 4 * ci + b
                        nc.gpsimd.indirect_dma_start(
                            out=gd[:, b, :],
                            out_offset=None,
                            in_=tab[:, :],
                            in_offset=bass.IndirectOffsetOnAxis(
                                ap=sidx[:, q:q + 1], axis=0),
                        )
                    return gd

                def edge_pipeline(conv):
                    tab = tabs[conv - 1]
                    with (
                        tc.tile_pool(name=f"gp{conv}_{it_}", bufs=2) as gp,
                        tc.tile_pool(name=f"ms{conv}_{it_}", bufs=3) as msp,
                    ):
                        for r in range(KDEG):
                            if conv == 1:
                                for ci, (cs, cw) in enumerate(CHUNKS):
                                    csl = slice(cs, cs + cw)
                                    gd = gath(gp, tab, r, ci, cw)
                                    m = _edge_msgs(msp, gd, cs, cw)
                                    po = psO.tile([128, 512], f32, tag="po")
                                    mm(po[:, :cw], wt["m1W2"][:], m[:, :cw])
                                    nc.vector.tensor_tensor(
                                        out=acc[:, csl], in0=po[:, :cw],
                                        in1=acc[:, csl], op=ALU.max)
                            else:
                                for G in range(4):
                                    ccs = list(range(4 * G, min(4 * G + 4, NCH)))
                                    po = psO.tile([128, 512], f32, tag="po")
                                    for qd, ci in enumerate(ccs):
                                        cs, cw = CHUNKS[ci]
                                        gd = gath(gp, tab, r, ci, cw)
                                        m = _edge_msgs(msp, gd, cs, cw)
                                        mm(po[32 * qd:32 * qd + 32, :cw],
                                           wt["m2W2"][:], m[:, :cw],
                                           tile_position=(0, 32 * qd))
                                    gw = 512 if G < 3 else 128
                                    rsl = slice(0, 128) if G < 3 else slice(0, 4)
                                    gcol = slice(512 * G, 512 * G + gw)
                                    nc.vector.tensor_tensor(
                                        out=acc2v[rsl, gcol],
                                        in0=po[rsl, :gw],
                                        in1=acc2v[rsl, gcol], op=ALU.max)

                def _edge_msgs(msp, gd, cs, cw):
                    csl = slice(cs, cs + cw)
                    nb = cw // 128
                    pt = psT.tile([128, 512], fp16, tag="pt")
                    for b in range(nb):
                        nc.tensor.transpose(
                            out=pt[:, 128 * b:128 * (b + 1)],
                            in_=gd[:, b, :],
                            identity=identh[:],
                        )
                    m = msp.tile([128, 512], fp16, tag="m")
                    nc.vector.tensor_tensor(
                        out=m[:, :cw], in0=pt[:, :cw], in1=hcb[:, csl],
                        op=ALU.add)
                    nc.scalar.activation(m[:, :cw], m[:, :cw], AF.Relu)
                    return m

                # ---------- conv1 ----------
                build_hcb(1)
                # std(t) reciprocal during AllGather 1: full-row passes
                with tc.tile_pool(name=f"sst{it_}", bufs=2) as ssp:
                    for cs, cw in CHUNKS:
                        csl = slice(cs, cs + cw)
                        ss = ssp.tile([1, 1024], f32, tag="ss")
                        nc.scalar.activation(ss[0:1, 0:cw], bandt[0:1, csl],
                                             AF.Exp, scale=2.0 * LOG_SIGMA)
                        nc.vector.tensor_scalar(
                            out=ss[0:1, 512:512 + cw], in0=ss[0:1, 0:cw],
                            scalar1=-1.0, scalar2=1.0 / (2.0 * LOG_SIGMA),
                            op0=ALU.add, op1=ALU.mult)
                        nc.scalar.activation(ss[0:1, 0:cw],
                                             ss[0:1, 512:512 + cw], AF.Sqrt)
                        nc.vector.tensor_scalar_add(
                            out=ss[0:1, 512:512 + cw], in0=ss[0:1, 0:cw],
                            scalar1=1e-7)
                        with nc.allow_low_precision(
                                reason="fp16 recip; 0.05%% ok at 2e-2 tol"):
                            nc.vector.reciprocal(out=bandb[96:97, csl],
                                                 in_=ss[0:1, 512:512 + cw])

                edge_pipeline(1)
                # out1 = relu(acc + m1b2) -> xT0 (conv2 input)
                for cs, cw in CHUNKS:
                    csl = slice(cs, cs + cw)
                    nc.scalar.activation(xT0[:, csl], acc[:, csl], AF.Relu,
                                         bias=m1b2)

                if debug:
                    nc.sync.dma_start(out=dbg["dxT1"][:, :], in_=xT1[:])
                    nc.sync.dma_start(out=dbg["dxT2"][:, :], in_=xT2[:])
                    nc.sync.dma_start(out=dbg["dhcb"][:, :], in_=hcb[:])
                    nc.sync.dma_start(out=dbg["dout1"][:, :], in_=xT0[:])
                    nc.sync.dma_start(out=dbg["dtabL"][:, :],
                                      in_=tabs[0][0:128, :])
                    nc.sync.dma_start(out=dbg["dtabH"][:, :],
                                      in_=tabs[0][HALF:HALF + 128, :])
                    nc.sync.dma_start(out=dbg["drcp"][:, :], in_=bandb[96:97, :])

                # ---------- conv2 ----------
                with tc.tile_pool(name=f"t2p{it_}", bufs=2) as t2p:
                    build_table2()
                nc.gpsimd.collective_compute(
                    "AllGather", ALU.bypass, replica_groups=RG,
                    ins=[inb[1].ap().opt()],
                    outs=[tabs[1].ap().opt()],
                )
                build_hcb(2)
                edge_pipeline(2)

                if debug:
                    nc.sync.dma_start(out=dbg["dacc2"][:, :], in_=acc2v[:])

                # ---------- final scale: y = (acc2 + m2b2) * recip ----------
                for G in range(4):
                    ccs = list(range(4 * G, min(4 * G + 4, NCH)))
                    pr = psA.tile([128, 512], f32, tag="pa")
                    gw = 512 if G < 3 else 128
                    for qd, ci in enumerate(ccs):
                        cs, cw = CHUNKS[ci]
                        mm(pr[32 * qd:32 * qd + 32, :cw],
                           smb[96:97, 128:160], bandb[96:97, cs:cs + cw],
                           tile_position=(96, 32 * qd))
                    gcol = slice(512 * G, 512 * G + gw)
                    rsl = slice(0, 128) if G < 3 else slice(0, 4)
                    nc.vector.scalar_tensor_tensor(
                        out=acc2v[rsl, gcol], in0=acc2v[rsl, gcol],
                        scalar=m2b2v if G < 3 else bt[0:4, 8:9],
                        in1=pr[rsl, :gw],
                        op0=ALU.add, op1=ALU.mult)
                nc.sync.dma_start(out=y_out[:, :], in_=acc2v[:])

    _split_multi_waits(nc, mybir)
    return nc


def _host_prep(inputs):
    """Build per-core input maps from full inputs."""
    t = np.asarray(inputs["t"], np.float32).reshape(N)
    obj_x = np.asarray(inputs["obj_x"], np.float32)
    obj_geo = np.asarray(inputs["obj_geo"], np.float32)
    wall = np.asarray(inputs["wall"], np.float32)
    category = np.asarray(inputs["category"]).astype(np.int64)
    batch_idx = np.asarray(inputs["batch_idx"]).astype(np.int64)
    src = np.asarray(inputs["src"]).astype(np.int64)
    dst = np.asarray(inputs["dst"]).astype(np.int64)

    # edge slots: slot (r, n) holds the r-th incoming edge of node n
    if np.array_equal(dst, np.tile(np.arange(N, dtype=dst.dtype), E // N)):
        src_slots = src.reshape(KDEG, N)
    else:
        order = np.argsort(dst, kind="stable")
        counts = np.bincount(dst, minlength=N)
        assert (counts == KDEG).all(), "kernel requires uniform in-degree 10"
        src_slots = np.empty((KDEG, N), np.int64)
        srt = src[order].reshape(N, KDEG)
        src_slots[:, :] = srt.T

    wall_pn = wall[batch_idx]  # [N, 2]

    def f16c(x):
        return np.ascontiguousarray(np.asarray(x, np.float16))

    def f32c(x):
        return np.ascontiguousarray(np.asarray(x, np.float32))

    smb = np.zeros((97, 256), np.float16)
    smb[0:6, 0:128] = f16c(inputs["i1"])
    smb[32:42, 0:64] = f16c(inputs["embed_W"])
    smb[64:66, 64:128] = f16c(inputs["w1"])

    gw = np.asarray(inputs["gfp_W"], np.float64).reshape(32)
    smb[96, 128:132] = 1.0    # ones4 for the recip broadcast matmul

    btile = np.zeros((128, 16), np.float32)
    btile[:, 0] = f32c(inputs["ib1"])
    btile[:, 1] = f32c(inputs["ib2"])
    btile[64:128, 2] = f32c(inputs["sb"])
    btile[64:128, 3] = f32c(inputs["wb1"])
    btile[0:64, 4] = f32c(inputs["wb2"])
    btile[:, 5] = f32c(inputs["m1b1"])
    btile[:, 6] = f32c(inputs["m1b2"])
    btile[:, 7] = f32c(inputs["m2b1"])
    b2 = f32c(inputs["m2b2"])
    for q in range(4):
        btile[32 * q:32 * q + 4, 8] = b2

    m1W1 = f32c(inputs["m1W1"])
    m2W1 = f32c(inputs["m2W1"])
    m1c = m1W1[:320] - m1W1[320:]
    m1n = m1W1[320:]
    m2c = m2W1[:320] - m2W1[320:]
    m2n = m2W1[320:]

    wmap = {
        "i2": f16c(inputs["i2"]), "sW": f16c(inputs["sW"]),
        "w2": np.concatenate(
            [np.zeros((64, 64), np.float16), f16c(inputs["w2"])], axis=0),
        "m1W2": f16c(inputs["m1W2"]),
        "m2W2": np.concatenate(
            [f16c(inputs["m2W2"]), np.zeros((128, 28), np.float16)], axis=1),
        "m1cA": f16c(m1c[0:128]), "m1cB": f16c(m1c[128:256]),
        "m1cC": f16c(m1c[256:320]),
        "m1nA": f16c(m1n[0:128]), "m1nB": f16c(m1n[128:256]),
        "m1nC": f16c(m1n[256:320]),
        "m2cA": f16c(m2c[0:128]), "m2cB": f16c(m2c[128:256]),
        "m2cC": f16c(m2c[256:320]),
        "m2nA": f16c(m2n[0:128]), "m2nB": f16c(m2n[128:256]),
        "m2nC": f16c(m2n[256:320]),
    }
    wmap = {k: np.ascontiguousarray(v) for k, v in wmap.items()}

    in_maps = []
    for c in range(N_CORES):
        n0 = c * SH
        nreal = min(max(N - n0, 0), SH)
        bandb = np.zeros((97, SH), np.float16)
        bandb[0:4, :nreal] = obj_x[n0:n0 + nreal].T
        bandb[4:6, :nreal] = obj_geo[n0:n0 + nreal].T
        cat = category[n0:n0 + nreal]
        bandb[32 + cat, np.arange(nreal)] = 1.0  # one-hot
        bandb[64:66, :nreal] = wall_pn[n0:n0 + nreal].T

        bandt = np.zeros((1, SH), np.float32)
        bandt[0, :nreal] = t[n0:n0 + nreal]
        bandt[0, nreal:] = 1.0

        # centered fractional phases of t*w (and +1/4 for the cos half):
        # sin(2*pi*t*w) == sin(2*pi*m), m in [-0.5, 0.5]
        tw = t[n0:n0 + nreal].astype(np.float64)[None, :] * gw[:, None]
        gfpm = np.zeros((64, SH), np.float16)
        ms = (tw + 0.5) % 1.0 - 0.5
        mc = (tw + 0.75) % 1.0 - 0.5
        gfpm[0:32, :nreal] = ms.astype(np.float16)
        gfpm[32:64, :nreal] = mc.astype(np.float16)

        # int32 gather indices: column r*NBLK+b, partition p -> src of
        # node 128*b + p in round r
        sl = src_slots[:, n0:n0 + nreal].astype(np.int64)  # [10, nreal]
        sfull = np.zeros((KDEG, SH), np.int64)
        sfull[:, :nreal] = sl
        sidx = (sfull.reshape(KDEG, NBLK, 128).transpose(2, 0, 1)
                .reshape(128, KDEG * NBLK)).astype(np.int32)

        im = {
            "bandb": bandb,
            "bandt": bandt,
            "gfpm": gfpm,
            "sidx": np.ascontiguousarray(sidx),
            "smb": smb,
            "btile": btile,
        }
        im.update(wmap)
        in_maps.append(im)
    return in_maps


def _unshard(results):
    out = np.empty((NPAD, 4), np.float32)
    for c in range(N_CORES):
        yv = results[c]["y"]  # [128, 1664]
        for cc in range(NCH):
            G, q = cc // 4, cc % 4
            cs, cw = CHUNKS[cc]
            out[c * SH + cs: c * SH + cs + cw, :] = (
                yv[32 * q:32 * q + 4, 512 * G:512 * G + cw].T
            )
    return out[:N]


def kernel(**inputs) -> np.ndarray:
    from concourse.bass_utils import run_bass_kernel_spmd

    if "nc" not in _CACHE:
        _CACHE["nc"] = _build()
    nc = _CACHE["nc"]
    in_maps = _host_prep(inputs)
    import time as _time
    last_err = None
    for attempt in range(4):
        try:
            res = run_bass_kernel_spmd(nc, in_maps,
                                       core_ids=list(range(N_CORES)))
            break
        except Exception as e:  # transient NRT device wedge recovers on retry
            last_err = e
            _time.sleep(15 * (attempt + 1))
    else:
        raise last_err
    _CACHE["last_results"] = res
    return _unshard(res.results)


import concourse.bass as bass  # noqa: E402  (used inside _build closures)
